# revision 36
# baseline (speedup 1.0000x reference)
"""Graphormer layer on 8 TRN2 NeuronCores.

Sharding: core c handles batch b = c//2 and query-row half qh = c%2 (1024 q
rows). All compute is in transposed (feature-on-partition) layout; the host
pre-transposes x and the influence factor matrices and transposes per-core
outputs back during the gather. Host also rolls the node axis per core so
each core's own q rows sit at columns [0, 1024) — the device program is
identical across cores.

Host precomputes (bf16): EG = exp(iw1*u + ib1)/16 and GB = EG*(iw2*u + ib2),
so the device needs no influence prep and no psum preloads:
  per (qc, kc, half, st):
    psum = KT.T @ QT      (bf16, row-packed, d=32 per head)
    e    = exp(psum)      (ACT -> bf16)
    zsrc = e * EG[kc]     (DVE bf16;   Z  += ones.T @ zsrc, col-packed)
    f    = e * GB[kc]     (DVE bf16;   WV += V.T @ f,       col-packed)
  attn = (WV / Z) @ Wo + bo ; h = attn + xT_own
  out = W2.T-proj(relu(W1.T-proj(LN_T(h)) + b1)) + b2 + h
"""

import math

import numpy as np
import ml_dtypes

import concourse.bass as bass
import concourse.bacc as bacc
import concourse.mybir as mybir
import concourse.tile as tile
from concourse.bass_utils import run_bass_kernel_spmd

B, N, E, H, D = 4, 2048, 256, 8, 32
NQ = N // 2          # q rows per core
QC = 512             # q window
NKC = N // 128       # 16 k-chunks
EC = E // 128        # 2 feature chunks

f32 = mybir.dt.float32
bf16 = mybir.dt.bfloat16
FT = mybir.ActivationFunctionType
ALU = mybir.AluOpType

# vecs_sb column index: vec v, chunk c -> 2*v + c
V_G1, V_BETA1, V_G2, V_BETA2, V_BO, V_B1, V_B2 = range(7)


def layer_norm_T(nc, pp, ps, x_chunks, win, wn, g_col, beta_col, vecs, ones,
                 eps_ap, out_chunks, fast16=False, ptag=None, ones16=None):
    """LayerNorm over the partition dim (E = 2 chunks) in T layout.

    fast16: bf16 ones-matmul sums (2x PE rate; stats precision ~bf16).
    """
    sq = ps.tile([128, 2 * wn], bf16, name="lnsq", tag="lnsq")
    p_s = pp.tile([128, wn], f32, name="lnps", tag=ptag or "lnps")
    p_sq = pp.tile([128, wn], f32, name="lnpsq", tag=ptag or "lnpsq")
    if fast16:
        xb = ps.tile([128, 2 * wn], bf16, name="lnxb", tag="lnxb")
        for c in range(EC):
            nc.vector.tensor_copy(xb[:, c * wn:(c + 1) * wn],
                                  x_chunks[c][:, win:win + wn])
        xsum = [xb[:, 0:wn], xb[:, wn:2 * wn]]
    else:
        xsum = [x_chunks[c][:, win:win + wn] for c in range(EC)]
    for c in range(EC):
        xs = x_chunks[c][:, win:win + wn]
        nc.vector.tensor_mul(sq[:, c * wn:(c + 1) * wn], xs, xs)
        nc.tensor.matmul(p_s[:, :], ones[:, :], xsum[c],
                         start=(c == 0), stop=(c == EC - 1))
    for c in range(EC):
        nc.tensor.matmul(p_sq[:, :], ones16[:, :],
                         sq[:, c * wn:(c + 1) * wn],
                         start=(c == 0), stop=(c == EC - 1))
    mu = ps.tile([128, wn], f32, name="lnmu", tag="lnmu")
    msq = ps.tile([128, wn], f32, name="lnmsq", tag="lnmsq")
    nc.vector.tensor_scalar_mul(mu[:, :], p_s[:, :], 1.0 / E)
    nc.vector.tensor_scalar_mul(msq[:, :], p_sq[:, :], 1.0 / E)
    mu2 = ps.tile([128, wn], f32, name="lnmu2", tag="lnmu2")
    nc.vector.tensor_mul(mu2[:, :], mu[:, :], mu[:, :])
    var = ps.tile([128, wn], f32, name="lnvar", tag="lnvar")
    nc.vector.tensor_sub(var[:, :], msq[:, :], mu2[:, :])
    sd = ps.tile([128, wn], f32, name="lnsd", tag="lnsd")
    nc.scalar.activation(sd[:, :], var[:, :], FT.Sqrt, bias=eps_ap)
    rstd = ps.tile([128, wn], f32, name="lnrstd", tag="lnrstd")
    nc.vector.reciprocal_approx_fast(rstd[:, :], sd[:, :])
    for c in range(EC):
        xs = x_chunks[c][:, win:win + wn]
        xm = ps.tile([128, wn], f32, name="lnxm", tag="lnxm")
        nc.vector.tensor_sub(xm[:, :], xs, mu[:, :])
        xm2 = ps.tile([128, wn], f32, name="lnxm2", tag="lnxm2")
        nc.vector.tensor_mul(xm2[:, :], xm[:, :], rstd[:, :])
        nc.vector.tensor_scalar(
            out_chunks[c][:, win:win + wn], xm2[:, :],
            vecs[:, 2 * g_col + c:2 * g_col + c + 1],
            vecs[:, 2 * beta_col + c:2 * beta_col + c + 1],
            ALU.mult, ALU.add)


def build_body(nc, tc, xT_d, eg_d, gb_d, w_d, vecs_d, outT_d):
    persist_pools = []

    def ppool(name):
        p = tc.tile_pool(name=name, bufs=1)
        persist_pools.append(p)
        return p.__enter__()

    persist = ppool("persist")

    # ---- persistent SBUF ----
    qt = [persist.tile([128, NQ], bf16, name=f"qt{c}", tag=f"qt{c}") for c in range(EC)]
    kt = [persist.tile([128, N], bf16, name=f"kt{c}", tag=f"kt{c}") for c in range(EC)]
    xtq = [persist.tile([128, NQ], f32, name=f"xtq{c}", tag=f"xtq{c}") for c in range(EC)]
    v_sb = [persist.tile([128, E], bf16, name=f"v{k}", tag=f"v{k}") for k in range(NKC)]
    ga_sb = [persist.tile([128, NQ], bf16, name=f"ga_{k}", tag=f"ga_{k}") for k in range(NKC)]
    gb_sb = [persist.tile([128, NQ], bf16, name=f"gb_{k}", tag=f"gb_{k}") for k in range(NKC)]
    w_bf = {n: persist.tile([128, 2 * E], bf16, name=f"wbf_{n}", tag=f"wbf_{n}")
            for n in w_d}
    vecs = persist.tile([128, 14], f32, name="vecs", tag="vecs")
    ones = persist.tile([128, 128], f32, name="ones", tag="ones")
    ones_b128 = persist.tile([128, 128], bf16, name="ones_b128", tag="ones_b128")
    ones_bf = persist.tile([128, 32], bf16, name="ones_bf", tag="ones_bf")
    h_sb = [[persist.tile([128, QC], f32, name=f"h{q}{c}", tag=f"h{q}{c}") for c in range(EC)]
            for q in range(2)]
    eps_t = persist.tile([128, 1], f32, name="eps_t", tag="eps_t")

    # ---- loads: weights -> xT -> influence factors (DMA streams early) ----
    with tc.tile_pool(name="wload", bufs=1) as wl:
        w_sb = {}
        for n in w_d:
            w_sb[n] = wl.tile([128, 2 * E], f32, name=f"w_{n}", tag=f"w_{n}")
            for c in range(EC):
                nc.sync.dma_start(w_sb[n][:, E * c:E * (c + 1)],
                                  w_d[n][128 * c:128 * (c + 1), :])
        nc.sync.dma_start(vecs[:, :], vecs_d[:, :])

        with tc.tile_pool(name="xt_pool", bufs=1) as xp:
            xt = [xp.tile([128, N], f32, name=f"xt{c}", tag=f"xt{c}")
                  for c in range(EC)]
            for c in range(EC):
                nc.sync.dma_start(xt[c][:, :], xT_d[128 * c:128 * (c + 1), :])
            for k in range(NKC):
                nc.sync.dma_start(ga_sb[k][:, :], eg_d[128 * k:128 * (k + 1), :])
                nc.sync.dma_start(gb_sb[k][:, :], gb_d[128 * k:128 * (k + 1), :])

            # consts
            nc.vector.memset(eps_t[:, :], 1e-5)
            nc.vector.memset(ones[:, :], 1.0)
            nc.vector.memset(ones_b128[:, :], 1.0)
            nc.vector.memset(ones_bf[:, :], 1.0)
            for n in w_d:
                nc.vector.tensor_copy(w_bf[n][:, :], w_sb[n][:, :])
            for c in range(EC):
                nc.vector.tensor_copy(xtq[c][:, :], xt[c][:, :NQ])

            # ---- LN1 + projections ----
            with tc.tile_pool(name="ln_psum", bufs=2, space="PSUM") as ln_pp, \
                 tc.tile_pool(name="ln_sbuf", bufs=2) as ln_ps, \
                 tc.tile_pool(name="proj_psum", bufs=2, space="PSUM") as proj_psum:
                ln1 = [xp.tile([128, N], bf16, name=f"ln1{c}", tag=f"ln1{c}")
                       for c in range(EC)]
                for w in range(N // 512):
                    layer_norm_T(nc, ln_pp, ln_ps, xt, 512 * w, 512, V_G1,
                                 V_BETA1, vecs, ones, eps_t[:, :], ln1,
                                 ones16=ones_b128)
                for fc in range(EC):
                    for qw in range(NQ // 512):
                        pq = proj_psum.tile([128, 512], f32, name="proj", tag="proj")
                        for ec in range(EC):
                            nc.tensor.matmul(
                                pq[:, :],
                                w_bf["Wq"][:, E * ec + 128 * fc:E * ec + 128 * (fc + 1)],
                                ln1[ec][:, 512 * qw:512 * (qw + 1)],
                                start=(ec == 0), stop=(ec == EC - 1))
                        nc.scalar.activation(qt[fc][:, 512 * qw:512 * (qw + 1)],
                                             pq[:, :], FT.Copy)
                for fc in range(EC):
                    for kw in range(N // 512):
                        pk = proj_psum.tile([128, 512], f32, name="proj", tag="proj")
                        for ec in range(EC):
                            nc.tensor.matmul(
                                pk[:, :],
                                w_bf["Wk"][:, E * ec + 128 * fc:E * ec + 128 * (fc + 1)],
                                ln1[ec][:, 512 * kw:512 * (kw + 1)],
                                start=(ec == 0), stop=(ec == EC - 1))
                        nc.scalar.activation(kt[fc][:, 512 * kw:512 * (kw + 1)],
                                             pk[:, :], FT.Copy)
                for k in range(NKC):
                    pv = proj_psum.tile([128, E], f32, name="projv", tag="projv")
                    for ec in range(EC):
                        nc.tensor.matmul(
                            pv[:, :],
                            ln1[ec][:, 128 * k:128 * (k + 1)],
                            w_bf["Wv"][:, E * ec:E * (ec + 1)],
                            start=(ec == 0), stop=(ec == EC - 1))
                    nc.scalar.activation(v_sb[k][:, :], pv[:, :], FT.Copy)

    # ---- attention + FFN per q window (shared pools so FFN(qc0)
    #      overlaps attention(qc1)) ----
    with tc.tile_pool(name="score_psum", bufs=2, space="PSUM") as sp, \
         tc.tile_pool(name="ef_sbuf", bufs=7) as efp, \
         tc.tile_pool(name="att_sbuf", bufs=2) as asb:
      for qc in range(2):
        q0 = QC * qc
        with tc.tile_pool(name="acc_psum", bufs=1, space="PSUM") as ap_:
            wv_ps = [ap_.tile([128, QC], f32, name=f"wv{s}", tag=f"wv{s}")
                     for s in range(2)]
            z_ps = [ap_.tile([128, QC], f32, name=f"z{s}", tag=f"z{s}")
                    for s in range(2)]
            for kc in range(NKC):
                gab = ga_sb[kc][:, q0:q0 + QC].rearrange(
                    "p (o q) -> p o q", o=1).broadcast_to([128, 2, QC])
                gbb = gb_sb[kc][:, q0:q0 + QC].rearrange(
                    "p (o q) -> p o q", o=1).broadcast_to([128, 2, QC])
                for half in range(2):  # head sets {0-3}, {4-7}
                    # 4 QK matmuls on distinct row-groups/banks -> 4-way pack
                    sts = [sp.tile([128, 2 * QC], f32, name="score",
                                   tag="score") for _ in range(2)]
                    for stg in range(2):
                        for j in range(2):
                            h = 4 * half + 2 * stg + j
                            hh = 32 * (h % 4)
                            nc.tensor.matmul(
                                sts[stg][:, QC * j:QC * (j + 1)],
                                kt[half][hh:hh + 32, 128 * kc:128 * (kc + 1)],
                                qt[half][hh:hh + 32, q0:q0 + QC],
                                start=True, stop=True,
                                skip_group_check=True, tile_position=(hh, 0))
                    zsrcs, fs_ = [], []
                    for stg in range(2):
                        st = sts[stg]
                        e = efp.tile([128, 2 * QC], bf16, name="e", tag="e")
                        nc.scalar.activation(e[:, :], st[:, :], FT.Exp)
                        er = e[:, :].rearrange("p (o q) -> p o q", o=2)
                        zsrc = efp.tile([128, 2 * QC], bf16, name="t", tag="t")
                        nc.vector.tensor_tensor(
                            zsrc[:, :].rearrange("p (o q) -> p o q", o=2),
                            er, gab, ALU.mult)
                        f = efp.tile([128, 2 * QC], bf16, name="f", tag="f")
                        nc.vector.tensor_tensor(
                            f[:, :].rearrange("p (o q) -> p o q", o=2),
                            er, gbb, ALU.mult)
                        zsrcs.append(zsrc)
                        fs_.append(f)
                    # 4 Z then 4 WV: runs of 4 distinct array columns -> pack
                    for stg in range(2):
                        for j in range(2):
                            h = 4 * half + 2 * stg + j
                            s_, hh = h // 4, 32 * (h % 4)
                            nc.tensor.matmul(
                                z_ps[s_][hh:hh + 32, :],
                                ones_bf[:, :],
                                zsrcs[stg][:, QC * j:QC * (j + 1)],
                                start=(kc == 0), stop=(kc == NKC - 1),
                                skip_group_check=True, tile_position=(0, hh))
                    for stg in range(2):
                        for j in range(2):
                            h = 4 * half + 2 * stg + j
                            s_, hh = h // 4, 32 * (h % 4)
                            nc.tensor.matmul(
                                wv_ps[s_][hh:hh + 32, :],
                                v_sb[kc][:, 32 * h:32 * h + 32],
                                fs_[stg][:, QC * j:QC * (j + 1)],
                                start=(kc == 0), stop=(kc == NKC - 1),
                                skip_group_check=True, tile_position=(0, hh))
            # normalize + Wo projection + bias + residual -> h
            on = []
            for s in range(2):
                zr = asb.tile([128, QC], f32, name=f"zr{s}", tag=f"zr{s}")
                nc.vector.reciprocal_approx_fast(zr[:, :], z_ps[s][:, :])
                o = asb.tile([128, QC], bf16, name=f"on{s}", tag=f"on{s}")
                nc.vector.tensor_mul(o[:, :], wv_ps[s][:, :], zr[:, :])
                on.append(o)
            for fc in range(EC):
                po = sp.tile([128, QC], f32, name="score", tag="score")
                for ec in range(EC):
                    nc.tensor.matmul(
                        po[:, :],
                        w_bf["Wo"][:, E * ec + 128 * fc:E * ec + 128 * (fc + 1)],
                        on[ec][:, :],
                        start=(ec == 0), stop=(ec == EC - 1))
                ta = asb.tile([128, QC], f32, name="tattn", tag="tattn")
                nc.vector.tensor_scalar_add(ta[:, :], po[:, :],
                                            vecs[:, 2 * V_BO + fc:2 * V_BO + fc + 1])
                nc.vector.tensor_add(h_sb[qc][fc][:, :], ta[:, :],
                                     xtq[fc][:, q0:q0 + QC])

        # ---- LN2 + FFN + residual + store (psum from the shared score
        #      pool so this overlaps the next q window's attention) ----
        with tc.tile_pool(name="ln_sbuf2", bufs=1) as ln_ps2, \
             tc.tile_pool(name="ffn_sbuf", bufs=2) as fsb:
            ln2 = [fsb.tile([128, QC], bf16, name=f"ln2{c}", tag=f"ln2{c}")
                   for c in range(EC)]
            layer_norm_T(nc, sp, ln_ps2, h_sb[qc], 0, QC, V_G2, V_BETA2,
                         vecs, ones, eps_t[:, :], ln2, ptag="score",
                         ones16=ones_b128)
            z1 = [fsb.tile([128, QC], bf16, name=f"z1{c}", tag=f"z1{c}")
                  for c in range(EC)]
            for fc in range(EC):
                p1 = sp.tile([128, QC], f32, name="ffn", tag="score")
                for ec in range(EC):
                    nc.tensor.matmul(
                        p1[:, :],
                        w_bf["W1"][:, E * ec + 128 * fc:E * ec + 128 * (fc + 1)],
                        ln2[ec][:, :],
                        start=(ec == 0), stop=(ec == EC - 1))
                nc.vector.tensor_scalar(z1[fc][:, :], p1[:, :],
                                        vecs[:, 2 * V_B1 + fc:2 * V_B1 + fc + 1],
                                        0.0, ALU.add, ALU.max)
            for fc in range(EC):
                p2 = sp.tile([128, QC], f32, name="ffn", tag="score")
                for ec in range(EC):
                    nc.tensor.matmul(
                        p2[:, :],
                        w_bf["W2"][:, E * ec + 128 * fc:E * ec + 128 * (fc + 1)],
                        z1[ec][:, :],
                        start=(ec == 0), stop=(ec == EC - 1))
                t2 = fsb.tile([128, QC], f32, name="t2", tag="t2")
                nc.vector.tensor_scalar_add(t2[:, :], p2[:, :],
                                            vecs[:, 2 * V_B2 + fc:2 * V_B2 + fc + 1])
                of = fsb.tile([128, QC], f32, name="of", tag="of")
                nc.vector.tensor_add(of[:, :], t2[:, :], h_sb[qc][fc][:, :])
                nc.sync.dma_start(
                    outT_d[128 * fc:128 * (fc + 1), QC * qc:QC * (qc + 1)],
                    of[:, :])

    for p in reversed(persist_pools):
        p.__exit__(None, None, None)


def build_nc():
    nc = bacc.Bacc(
        "TRN2",
        target_bir_lowering=False,
        debug=False,
        enable_asserts=False,
        num_devices=8,
    )
    xT_d = nc.dram_tensor("xT", [E, N], f32, kind="ExternalInput").ap()
    eg_d = nc.dram_tensor("egT", [N, NQ], bf16, kind="ExternalInput").ap()
    gb_d = nc.dram_tensor("gbT", [N, NQ], bf16, kind="ExternalInput").ap()
    w_d = {
        name: nc.dram_tensor(name, [E, E], f32, kind="ExternalInput").ap()
        for name in ("Wq", "Wk", "Wv", "Wo", "W1", "W2")
    }
    vecs_d = nc.dram_tensor("vecs", [128, 14], f32, kind="ExternalInput").ap()
    outT_d = nc.dram_tensor("outT", [E, NQ], f32, kind="ExternalOutput").ap()

    with tile.TileContext(nc) as tc:
        build_body(nc, tc, xT_d, eg_d, gb_d, w_d, vecs_d, outT_d)
    nc.compile()
    return nc


def host_shard(inputs):
    """Build the 8 per-core input maps (see module docstring for the roll)."""
    x = np.asarray(inputs["x"], np.float32)
    infl = np.asarray(inputs["influence_matrix"], np.float32)
    vec_list = ["g1", "beta1", "g2", "beta2", "bo", "b1", "b2"]
    vecs_np = np.empty((128, 14), np.float32)
    for vi, nm in enumerate(vec_list):
        v = np.asarray(inputs[nm], np.float32).reshape(E)
        vecs_np[:, 2 * vi] = v[:128]
        vecs_np[:, 2 * vi + 1] = v[128:]
    iw1 = float(np.asarray(inputs["iw1"])); ib1 = float(np.asarray(inputs["ib1"]))
    iw2 = float(np.asarray(inputs["iw2"])); ib2 = float(np.asarray(inputs["ib2"]))
    ws = {n: np.ascontiguousarray(np.asarray(inputs[n], np.float32))
          for n in ("Wq", "Wk", "Wv", "Wo", "W1", "W2")}
    ws["Wq"] = ws["Wq"] / math.sqrt(D)

    in_maps = []
    for core in range(8):
        b, qh = core // 2, core % 2
        qoff = qh * NQ
        xb = np.roll(x[b], -qoff, axis=0)          # [N, E], own rows first
        xT = np.ascontiguousarray(xb.T)            # [E, N]
        inf_slice = np.roll(infl[b][qoff:qoff + NQ, :], -qoff, axis=1)
        u = np.ascontiguousarray(inf_slice.T)      # [N(k), NQ]
        # EG = exp(LG)/16 and GB = EG*G2 (the 1/16 cancels in the division
        # and keeps zsrc/f comfortably scaled for bf16)
        eg = np.exp(iw1 * u + ib1 - math.log(16.0))
        egT = eg.astype(ml_dtypes.bfloat16)
        gbT = (eg * (iw2 * u + ib2)).astype(ml_dtypes.bfloat16)
        m = {"xT": xT, "egT": egT, "gbT": gbT, "vecs": vecs_np}
        m.update(ws)
        in_maps.append(m)
    return in_maps


_NC_CACHE = []


def kernel(**inputs):
    if not _NC_CACHE:
        _NC_CACHE.append(build_nc())
    nc = _NC_CACHE[0]
    in_maps = host_shard(inputs)
    res = run_bass_kernel_spmd(nc, in_maps, core_ids=list(range(8)))
    out = np.empty((B, N, E), np.float32)
    for core in range(8):
        b, qh = core // 2, core % 2
        out[b, qh * NQ:(qh + 1) * NQ, :] = np.asarray(
            res.results[core]["outT"], np.float32).T
    return out


# revision 37
# speedup vs baseline: 1.2111x; 1.2111x over previous
"""Graphormer layer on 8 TRN2 NeuronCores.

Sharding: core c handles batch b = c//2 and query-row half qh = c%2 (1024 q
rows). All compute is in transposed (feature-on-partition) layout; the host
pre-transposes x and the influence factor matrices and transposes per-core
outputs back during the gather. Host also rolls the node axis per core so
each core's own q rows sit at columns [0, 1024) — the device program is
identical across cores.

Host precomputes (bf16): EG = exp(iw1*u + ib1)/16 and GB = EG*(iw2*u + ib2),
so the device needs no influence prep and no psum preloads:
  per (qc, kc, half, st):
    psum = KT.T @ QT      (bf16, row-packed, d=32 per head)
    e    = exp(psum)      (ACT -> bf16)
    zsrc = e * EG[kc]     (DVE bf16;   Z  += ones.T @ zsrc, col-packed)
    f    = e * GB[kc]     (DVE bf16;   WV += V.T @ f,       col-packed)
  attn = (WV / Z) @ Wo + bo ; h = attn + xT_own
  out = W2.T-proj(relu(W1.T-proj(LN_T(h)) + b1)) + b2 + h
"""

import math

import numpy as np
import ml_dtypes

import concourse.bass as bass
import concourse.bacc as bacc
import concourse.mybir as mybir
import concourse.tile as tile
from concourse.bass_utils import run_bass_kernel_spmd

B, N, E, H, D = 4, 2048, 256, 8, 32
NQ = N // 2          # q rows per core
QC = 512             # q window
NKC = N // 128       # 16 k-chunks
EC = E // 128        # 2 feature chunks

f32 = mybir.dt.float32
bf16 = mybir.dt.bfloat16
FT = mybir.ActivationFunctionType
ALU = mybir.AluOpType

# vecs_sb column index: vec v, chunk c -> 2*v + c
V_G1, V_BETA1, V_G2, V_BETA2, V_BO, V_B1, V_B2 = range(7)


def layer_norm_T(nc, pp, ps, x_chunks, win, wn, g_col, beta_col, vecs, ones,
                 eps_ap, out_chunks, fast16=False, ptag=None, ones16=None):
    """LayerNorm over the partition dim (E = 2 chunks) in T layout.

    fast16: bf16 ones-matmul sums (2x PE rate; stats precision ~bf16).
    """
    sq = ps.tile([128, 2 * wn], bf16, name="lnsq", tag="lnsq")
    p_s = pp.tile([128, wn], f32, name="lnps", tag=ptag or "lnps")
    p_sq = pp.tile([128, wn], f32, name="lnpsq", tag=ptag or "lnpsq")
    if fast16:
        xb = ps.tile([128, 2 * wn], bf16, name="lnxb", tag="lnxb")
        for c in range(EC):
            nc.vector.tensor_copy(xb[:, c * wn:(c + 1) * wn],
                                  x_chunks[c][:, win:win + wn])
        xsum = [xb[:, 0:wn], xb[:, wn:2 * wn]]
    else:
        xsum = [x_chunks[c][:, win:win + wn] for c in range(EC)]
    for c in range(EC):
        xs = x_chunks[c][:, win:win + wn]
        nc.vector.tensor_mul(sq[:, c * wn:(c + 1) * wn], xs, xs)
        nc.tensor.matmul(p_s[:, :], ones[:, :], xsum[c],
                         start=(c == 0), stop=(c == EC - 1))
    for c in range(EC):
        nc.tensor.matmul(p_sq[:, :], ones16[:, :],
                         sq[:, c * wn:(c + 1) * wn],
                         start=(c == 0), stop=(c == EC - 1))
    mu = ps.tile([128, wn], f32, name="lnmu", tag="lnmu")
    msq = ps.tile([128, wn], f32, name="lnmsq", tag="lnmsq")
    nc.vector.tensor_scalar_mul(mu[:, :], p_s[:, :], 1.0 / E)
    nc.vector.tensor_scalar_mul(msq[:, :], p_sq[:, :], 1.0 / E)
    mu2 = ps.tile([128, wn], f32, name="lnmu2", tag="lnmu2")
    nc.vector.tensor_mul(mu2[:, :], mu[:, :], mu[:, :])
    var = ps.tile([128, wn], f32, name="lnvar", tag="lnvar")
    nc.vector.tensor_sub(var[:, :], msq[:, :], mu2[:, :])
    sd = ps.tile([128, wn], f32, name="lnsd", tag="lnsd")
    nc.scalar.activation(sd[:, :], var[:, :], FT.Sqrt, bias=eps_ap)
    rstd = ps.tile([128, wn], f32, name="lnrstd", tag="lnrstd")
    nc.vector.reciprocal_approx_fast(rstd[:, :], sd[:, :])
    for c in range(EC):
        xs = x_chunks[c][:, win:win + wn]
        xm = ps.tile([128, wn], f32, name="lnxm", tag="lnxm")
        nc.vector.tensor_sub(xm[:, :], xs, mu[:, :])
        xm2 = ps.tile([128, wn], f32, name="lnxm2", tag="lnxm2")
        nc.vector.tensor_mul(xm2[:, :], xm[:, :], rstd[:, :])
        nc.vector.tensor_scalar(
            out_chunks[c][:, win:win + wn], xm2[:, :],
            vecs[:, 2 * g_col + c:2 * g_col + c + 1],
            vecs[:, 2 * beta_col + c:2 * beta_col + c + 1],
            ALU.mult, ALU.add)


def build_body(nc, tc, xT_d, eg_d, gb_d, w_d, vecs_d, outT_d):
    persist_pools = []

    def ppool(name):
        p = tc.tile_pool(name=name, bufs=1)
        persist_pools.append(p)
        return p.__enter__()

    persist = ppool("persist")

    # ---- persistent SBUF ----
    qt = [persist.tile([128, NQ], bf16, name=f"qt{c}", tag=f"qt{c}") for c in range(EC)]
    kt = [persist.tile([128, N], bf16, name=f"kt{c}", tag=f"kt{c}") for c in range(EC)]
    xtq = [persist.tile([128, NQ], f32, name=f"xtq{c}", tag=f"xtq{c}") for c in range(EC)]
    v_sb = [persist.tile([128, E], bf16, name=f"v{k}", tag=f"v{k}") for k in range(NKC)]
    ga_sb = [persist.tile([128, NQ], bf16, name=f"ga_{k}", tag=f"ga_{k}") for k in range(NKC)]
    gb_sb = [persist.tile([128, NQ], bf16, name=f"gb_{k}", tag=f"gb_{k}") for k in range(NKC)]
    w_bf = {n: persist.tile([128, 2 * E], bf16, name=f"wbf_{n}", tag=f"wbf_{n}")
            for n in w_d}
    vecs = persist.tile([128, 14], f32, name="vecs", tag="vecs")
    ones = persist.tile([128, 128], f32, name="ones", tag="ones")
    ones_b128 = persist.tile([128, 128], bf16, name="ones_b128", tag="ones_b128")
    ones_bf = persist.tile([128, 32], bf16, name="ones_bf", tag="ones_bf")
    h_sb = [[persist.tile([128, QC], f32, name=f"h{q}{c}", tag=f"h{q}{c}") for c in range(EC)]
            for q in range(2)]
    eps_t = persist.tile([128, 1], f32, name="eps_t", tag="eps_t")

    # ---- loads: weights -> xT -> influence factors (DMA streams early) ----
    with tc.tile_pool(name="wload", bufs=1) as wl:
        w_sb = {}
        for n in w_d:
            w_sb[n] = wl.tile([128, 2 * E], f32, name=f"w_{n}", tag=f"w_{n}")
            for c in range(EC):
                nc.sync.dma_start(w_sb[n][:, E * c:E * (c + 1)],
                                  w_d[n][128 * c:128 * (c + 1), :])
        nc.sync.dma_start(vecs[:, :], vecs_d[:, :])

        with tc.tile_pool(name="xt_pool", bufs=1) as xp:
            xt = [xp.tile([128, N], f32, name=f"xt{c}", tag=f"xt{c}")
                  for c in range(EC)]
            for c in range(EC):
                nc.sync.dma_start(xt[c][:, :], xT_d[128 * c:128 * (c + 1), :])
            for k in range(NKC):
                nc.sync.dma_start(ga_sb[k][:, :], eg_d[128 * k:128 * (k + 1), :])
                nc.sync.dma_start(gb_sb[k][:, :], gb_d[128 * k:128 * (k + 1), :])

            # consts
            nc.vector.memset(eps_t[:, :], 1e-5)
            nc.vector.memset(ones[:, :], 1.0)
            nc.vector.memset(ones_b128[:, :], 1.0)
            nc.vector.memset(ones_bf[:, :], 1.0)
            for n in w_d:
                nc.vector.tensor_copy(w_bf[n][:, :], w_sb[n][:, :])
            for c in range(EC):
                nc.vector.tensor_scalar_add(
                    xtq[c][:, :], xt[c][:, :NQ],
                    vecs[:, 2 * V_BO + c:2 * V_BO + c + 1])

            # ---- LN1 + projections ----
            with tc.tile_pool(name="ln_psum", bufs=2, space="PSUM") as ln_pp, \
                 tc.tile_pool(name="ln_sbuf", bufs=2) as ln_ps, \
                 tc.tile_pool(name="proj_psum", bufs=2, space="PSUM") as proj_psum:
                ln1 = [xp.tile([128, N], bf16, name=f"ln1{c}", tag=f"ln1{c}")
                       for c in range(EC)]
                for w in range(N // 512):
                    layer_norm_T(nc, ln_pp, ln_ps, xt, 512 * w, 512, V_G1,
                                 V_BETA1, vecs, ones, eps_t[:, :], ln1,
                                 ones16=ones_b128)
                for fc in range(EC):
                    for qw in range(NQ // 512):
                        pq = proj_psum.tile([128, 512], f32, name="proj", tag="proj")
                        for ec in range(EC):
                            nc.tensor.matmul(
                                pq[:, :],
                                w_bf["Wq"][:, E * ec + 128 * fc:E * ec + 128 * (fc + 1)],
                                ln1[ec][:, 512 * qw:512 * (qw + 1)],
                                start=(ec == 0), stop=(ec == EC - 1))
                        nc.scalar.activation(qt[fc][:, 512 * qw:512 * (qw + 1)],
                                             pq[:, :], FT.Copy)
                for fc in range(EC):
                    for kw in range(N // 512):
                        pk = proj_psum.tile([128, 512], f32, name="proj", tag="proj")
                        for ec in range(EC):
                            nc.tensor.matmul(
                                pk[:, :],
                                w_bf["Wk"][:, E * ec + 128 * fc:E * ec + 128 * (fc + 1)],
                                ln1[ec][:, 512 * kw:512 * (kw + 1)],
                                start=(ec == 0), stop=(ec == EC - 1))
                        nc.scalar.activation(kt[fc][:, 512 * kw:512 * (kw + 1)],
                                             pk[:, :], FT.Copy)
                for k in range(NKC):
                    pv = proj_psum.tile([128, E], f32, name="projv", tag="projv")
                    for ec in range(EC):
                        nc.tensor.matmul(
                            pv[:, :],
                            ln1[ec][:, 128 * k:128 * (k + 1)],
                            w_bf["Wv"][:, E * ec:E * (ec + 1)],
                            start=(ec == 0), stop=(ec == EC - 1))
                    nc.scalar.activation(v_sb[k][:, :], pv[:, :], FT.Copy)

    # ---- attention + FFN per q window (shared pools so FFN(qc0)
    #      overlaps attention(qc1)) ----
    with tc.tile_pool(name="score_psum", bufs=2, space="PSUM") as sp, \
         tc.tile_pool(name="ef_sbuf", bufs=8) as efp, \
         tc.tile_pool(name="att_sbuf", bufs=2) as asb:
      for qc in range(2):
        q0 = QC * qc
        with tc.tile_pool(name="acc_psum", bufs=1, space="PSUM") as ap_:
            wv_ps = [ap_.tile([128, QC], f32, name=f"wv{s}", tag=f"wv{s}")
                     for s in range(2)]
            z_ps = [ap_.tile([128, QC], f32, name=f"z{s}", tag=f"z{s}")
                    for s in range(2)]
            for kc in range(NKC):
                gab = ga_sb[kc][:, q0:q0 + QC].rearrange(
                    "p (o q) -> p o q", o=1).broadcast_to([128, 2, QC])
                gbb = gb_sb[kc][:, q0:q0 + QC].rearrange(
                    "p (o q) -> p o q", o=1).broadcast_to([128, 2, QC])
                for half in range(2):  # head sets {0-3}, {4-7}
                    # 4 QK matmuls on distinct row-groups/banks -> 4-way pack
                    sts = [sp.tile([128, 2 * QC], f32, name="score",
                                   tag="score") for _ in range(2)]
                    for stg in range(2):
                        for j in range(2):
                            h = 4 * half + 2 * stg + j
                            hh = 32 * (h % 4)
                            nc.tensor.matmul(
                                sts[stg][:, QC * j:QC * (j + 1)],
                                kt[half][hh:hh + 32, 128 * kc:128 * (kc + 1)],
                                qt[half][hh:hh + 32, q0:q0 + QC],
                                start=True, stop=True,
                                skip_group_check=True, tile_position=(hh, 0))
                    zsrcs, fs_ = [], []
                    for stg in range(2):
                        st = sts[stg]
                        e = efp.tile([128, 2 * QC], bf16, name="e", tag="e")
                        nc.scalar.activation(e[:, :], st[:, :], FT.Exp)
                        er = e[:, :].rearrange("p (o q) -> p o q", o=2)
                        zsrc = efp.tile([128, 2 * QC], bf16, name="t", tag="t")
                        nc.vector.tensor_tensor(
                            zsrc[:, :].rearrange("p (o q) -> p o q", o=2),
                            er, gab, ALU.mult)
                        f = efp.tile([128, 2 * QC], bf16, name="f", tag="f")
                        nc.vector.tensor_tensor(
                            f[:, :].rearrange("p (o q) -> p o q", o=2),
                            er, gbb, ALU.mult)
                        zsrcs.append(zsrc)
                        fs_.append(f)
                    # 4 Z then 4 WV: runs of 4 distinct array columns -> pack
                    for stg in range(2):
                        for j in range(2):
                            h = 4 * half + 2 * stg + j
                            s_, hh = h // 4, 32 * (h % 4)
                            nc.tensor.matmul(
                                z_ps[s_][hh:hh + 32, :],
                                ones_bf[:, :],
                                zsrcs[stg][:, QC * j:QC * (j + 1)],
                                start=(kc == 0), stop=(kc == NKC - 1),
                                skip_group_check=True, tile_position=(0, hh))
                    for stg in range(2):
                        for j in range(2):
                            h = 4 * half + 2 * stg + j
                            s_, hh = h // 4, 32 * (h % 4)
                            nc.tensor.matmul(
                                wv_ps[s_][hh:hh + 32, :],
                                v_sb[kc][:, 32 * h:32 * h + 32],
                                fs_[stg][:, QC * j:QC * (j + 1)],
                                start=(kc == 0), stop=(kc == NKC - 1),
                                skip_group_check=True, tile_position=(0, hh))
            # normalize + Wo projection + bias + residual -> h
            on = []
            for s in range(2):
                zr = asb.tile([128, QC], f32, name=f"zr{s}", tag=f"zr{s}")
                nc.vector.reciprocal_approx_fast(zr[:, :], z_ps[s][:, :])
                o = asb.tile([128, QC], bf16, name=f"on{s}", tag=f"on{s}")
                nc.vector.tensor_mul(o[:, :], wv_ps[s][:, :], zr[:, :])
                on.append(o)
            for fc in range(EC):
                po = sp.tile([128, QC], f32, name="score", tag="score")
                for ec in range(EC):
                    nc.tensor.matmul(
                        po[:, :],
                        w_bf["Wo"][:, E * ec + 128 * fc:E * ec + 128 * (fc + 1)],
                        on[ec][:, :],
                        start=(ec == 0), stop=(ec == EC - 1))
                nc.vector.tensor_add(h_sb[qc][fc][:, :], po[:, :],
                                     xtq[fc][:, q0:q0 + QC])

        # ---- LN2 + FFN + residual + store (psum from the shared score
        #      pool so this overlaps the next q window's attention) ----
        with tc.tile_pool(name="ln_sbuf2", bufs=1) as ln_ps2, \
             tc.tile_pool(name="ffn_sbuf", bufs=2) as fsb:
            ln2 = [fsb.tile([128, QC], bf16, name=f"ln2{c}", tag=f"ln2{c}")
                   for c in range(EC)]
            layer_norm_T(nc, sp, ln_ps2, h_sb[qc], 0, QC, V_G2, V_BETA2,
                         vecs, ones, eps_t[:, :], ln2, ptag="score",
                         ones16=ones_b128)
            z1 = [fsb.tile([128, QC], bf16, name=f"z1{c}", tag=f"z1{c}")
                  for c in range(EC)]
            for fc in range(EC):
                p1 = sp.tile([128, QC], f32, name="ffn", tag="score")
                for ec in range(EC):
                    nc.tensor.matmul(
                        p1[:, :],
                        w_bf["W1"][:, E * ec + 128 * fc:E * ec + 128 * (fc + 1)],
                        ln2[ec][:, :],
                        start=(ec == 0), stop=(ec == EC - 1))
                nc.vector.tensor_scalar(z1[fc][:, :], p1[:, :],
                                        vecs[:, 2 * V_B1 + fc:2 * V_B1 + fc + 1],
                                        0.0, ALU.add, ALU.max)
            for fc in range(EC):
                p2 = sp.tile([128, QC], f32, name="ffn", tag="score")
                for ec in range(EC):
                    nc.tensor.matmul(
                        p2[:, :],
                        w_bf["W2"][:, E * ec + 128 * fc:E * ec + 128 * (fc + 1)],
                        z1[ec][:, :],
                        start=(ec == 0), stop=(ec == EC - 1))
                t2 = fsb.tile([128, QC], f32, name="t2", tag="t2")
                nc.vector.tensor_scalar_add(t2[:, :], p2[:, :],
                                            vecs[:, 2 * V_B2 + fc:2 * V_B2 + fc + 1])
                of = fsb.tile([128, QC], f32, name="of", tag="of")
                nc.vector.tensor_add(of[:, :], t2[:, :], h_sb[qc][fc][:, :])
                nc.sync.dma_start(
                    outT_d[128 * fc:128 * (fc + 1), QC * qc:QC * (qc + 1)],
                    of[:, :])

    for p in reversed(persist_pools):
        p.__exit__(None, None, None)


def build_nc():
    nc = bacc.Bacc(
        "TRN2",
        target_bir_lowering=False,
        debug=False,
        enable_asserts=False,
        num_devices=8,
    )
    xT_d = nc.dram_tensor("xT", [E, N], f32, kind="ExternalInput").ap()
    eg_d = nc.dram_tensor("egT", [N, NQ], bf16, kind="ExternalInput").ap()
    gb_d = nc.dram_tensor("gbT", [N, NQ], bf16, kind="ExternalInput").ap()
    w_d = {
        name: nc.dram_tensor(name, [E, E], f32, kind="ExternalInput").ap()
        for name in ("Wq", "Wk", "Wv", "Wo", "W1", "W2")
    }
    vecs_d = nc.dram_tensor("vecs", [128, 14], f32, kind="ExternalInput").ap()
    outT_d = nc.dram_tensor("outT", [E, NQ], f32, kind="ExternalOutput").ap()

    with tile.TileContext(nc) as tc:
        build_body(nc, tc, xT_d, eg_d, gb_d, w_d, vecs_d, outT_d)
    nc.compile()
    return nc


def host_shard(inputs):
    """Build the 8 per-core input maps (see module docstring for the roll)."""
    x = np.asarray(inputs["x"], np.float32)
    infl = np.asarray(inputs["influence_matrix"], np.float32)
    vec_list = ["g1", "beta1", "g2", "beta2", "bo", "b1", "b2"]
    vecs_np = np.empty((128, 14), np.float32)
    for vi, nm in enumerate(vec_list):
        v = np.asarray(inputs[nm], np.float32).reshape(E)
        vecs_np[:, 2 * vi] = v[:128]
        vecs_np[:, 2 * vi + 1] = v[128:]
    iw1 = float(np.asarray(inputs["iw1"])); ib1 = float(np.asarray(inputs["ib1"]))
    iw2 = float(np.asarray(inputs["iw2"])); ib2 = float(np.asarray(inputs["ib2"]))
    ws = {n: np.ascontiguousarray(np.asarray(inputs[n], np.float32))
          for n in ("Wq", "Wk", "Wv", "Wo", "W1", "W2")}
    ws["Wq"] = ws["Wq"] / math.sqrt(D)

    in_maps = []
    for core in range(8):
        b, qh = core // 2, core % 2
        qoff = qh * NQ
        xb = np.roll(x[b], -qoff, axis=0)          # [N, E], own rows first
        xT = np.ascontiguousarray(xb.T)            # [E, N]
        inf_slice = np.roll(infl[b][qoff:qoff + NQ, :], -qoff, axis=1)
        u = np.ascontiguousarray(inf_slice.T)      # [N(k), NQ]
        # EG = exp(LG)/16 and GB = EG*G2 (the 1/16 cancels in the division
        # and keeps zsrc/f comfortably scaled for bf16)
        eg = np.exp(iw1 * u + ib1 - math.log(16.0))
        egT = eg.astype(ml_dtypes.bfloat16)
        gbT = (eg * (iw2 * u + ib2)).astype(ml_dtypes.bfloat16)
        m = {"xT": xT, "egT": egT, "gbT": gbT, "vecs": vecs_np}
        m.update(ws)
        in_maps.append(m)
    return in_maps


_NC_CACHE = []


def kernel(**inputs):
    if not _NC_CACHE:
        _NC_CACHE.append(build_nc())
    nc = _NC_CACHE[0]
    in_maps = host_shard(inputs)
    res = run_bass_kernel_spmd(nc, in_maps, core_ids=list(range(8)))
    out = np.empty((B, N, E), np.float32)
    for core in range(8):
        b, qh = core // 2, core % 2
        out[b, qh * NQ:(qh + 1) * NQ, :] = np.asarray(
            res.results[core]["outT"], np.float32).T
    return out
